# revision 54
# baseline (speedup 1.0000x reference)
"""Causal self-attention (B=4, L=2048, D=1024, H=16) on 8 Trainium2 NeuronCores.

Sharding: core c -> (batch b = c//2, head-group g = c%2 of 8 heads).
Each core computes qkv projection for its 8 heads, causal attention, and a
partial out-projection (its head-group's rows of W_out). The host sums the
two partials per batch and adds biases (exact: out-proj is linear and the
v-bias passes through softmax-weighted averaging).

Pipeline (vs the sequential baseline, ~407us -> ~294us body):
  - QKV projection is split into 4 L-quarters and software-pipelined with
    attention: proj(quarter k+1) MMs interleave into attn(q-block k), so the
    PE fills the gaps left by the ACT-bound exp chain.
  - Causal narrowing: for diagonal k-tiles only the valid q-range is
    exp'd / AV-matmul'd (saves ~15% ACT + PE); the in-tile triangle is
    masked with a [128,128] mask multiply (10x less DVE mask work).
  - Global emission pipeline: scores are emitted 2 steps ahead across pair
    and q-block boundaries so exp never waits on filler matmuls; exp and
    mask are high-priority (they gate the AV accumulation chain).
  - bf16 everywhere except PSUM/normalization: x, W, q, k, v, exp output,
    o (rel err ~3.5e-3 vs the 2e-2 gate). PE runs 1 cyc/row either way
    (fp32r N>=256 == bf16), but bf16 halves DMA + SBUF and avoids the
    fp32r 4x penalty on narrowed N=128 diagonal scores.
  - Head: wq/xt-quarter-0 DMAs interleaved per-co so the first co
    accumulation starts after ~2 DMAs; wk/wv/wo stream behind compute.
  - Normalization multiplies read po directly from PSUM (no bounce copy).

Attention layout (transpose-free):
  qT, kT   [64d x L]  per head (2 heads stacked per 128 partitions)
  S^T tile [128k x 512q] = kT_tile.T @ qT_block   (PE, K=64, 2 heads
           concurrently via tile_position row groups)
  expS     = exp(S^T)  (ACT, PSUM->SBUF bf16), diagonal tiles narrowed
  O^T,sums [65 x 512q] += [V_tile | ones].T-form @ expS  (PE, K=128)
  O^T_norm = PSUM(O^T) * broadcast(1/sums)  -> bf16 lhsT of out-proj
  Y tile   [128l x 512e] = sum_pairs O^T_pair.T @ Wo_pair
"""

import os
from contextlib import ExitStack

import numpy as np

os.environ.setdefault("JAX_PLATFORMS", "")

import concourse.bass as bass
import concourse.mybir as mybir
import concourse.tile as tile
from concourse import bacc, bass_utils

F32 = mybir.dt.float32
F32R = mybir.dt.float32r
BF16 = mybir.dt.bfloat16
AF = mybir.ActivationFunctionType

B, L, D, H = 4, 2048, 1024, 16
DK = D // H            # 64
G = 2                  # head groups (tensor parallel)
HPG = H // G           # 8 heads per group
GW = HPG * DK          # 512 columns per group
P = 128
CO = D // P            # 8 contraction tiles for projections
LT = L // P            # 16 l-tiles / k-tiles
QW = 512               # q-block width
QB = L // QW           # 4 q-blocks
NPAIR = HPG // 2       # 4 head-pairs per group (2 heads per 128 partitions)

_NC_CACHE: dict = {}


def build_nc(with_qk_bias: bool, repeat: int = 1, ablate: frozenset = frozenset()):
    nc = bacc.Bacc("TRN2", target_bir_lowering=False, debug=False, num_devices=8)

    xt = nc.dram_tensor("xt", [D, L], BF16, kind="ExternalInput").ap()
    wq = nc.dram_tensor("wq", [D, GW], BF16, kind="ExternalInput").ap()
    wk = nc.dram_tensor("wk", [D, GW], BF16, kind="ExternalInput").ap()
    wv = nc.dram_tensor("wv", [D, GW], BF16, kind="ExternalInput").ap()
    wo = nc.dram_tensor("wo", [GW, D], BF16, kind="ExternalInput").ap()
    mtri = nc.dram_tensor("mtri", [P, P], BF16, kind="ExternalInput").ap()
    onesd = nc.dram_tensor("onesd", [P, 1], BF16, kind="ExternalInput").ap()
    if with_qk_bias:
        bq = nc.dram_tensor("bq", [P, NPAIR], F32, kind="ExternalInput").ap()
        bk = nc.dram_tensor("bk", [P, NPAIR], F32, kind="ExternalInput").ap()
    y = nc.dram_tensor("y", [L, D], F32, kind="ExternalOutput").ap()

    xt_r = xt.rearrange("(co p) l -> co p l", p=P)
    wq_r = wq.rearrange("(co p) c -> co p c", p=P)
    wk_r = wk.rearrange("(co p) c -> co p c", p=P)
    wv_r = wv.rearrange("(co p) c -> co p c", p=P)
    wo_r = wo.rearrange("(pr p) e -> pr p e", p=P)
    y_r = y.rearrange("(lt p) e -> lt p e", p=P)

    def mm(out, lhsT, rhs, start, stop, tile_position=None):
        nc.tensor.matmul(out, lhsT, rhs, start=start, stop=stop,
                         tile_position=tile_position)

    with tile.TileContext(nc) as tc, ExitStack() as ctx:
        constp = ctx.enter_context(tc.tile_pool(name="const", bufs=1))
        mtri_sb = constp.tile([P, P], BF16)
        ones_sb = constp.tile([P, 1], BF16)
        nc.sync.dma_start(mtri_sb[:], mtri)
        nc.sync.dma_start(ones_sb[:], onesd)
        if with_qk_bias:
            bq_sb = constp.tile([P, NPAIR], F32)
            bk_sb = constp.tile([P, NPAIR], F32)
            nc.sync.dma_start(bq_sb[:], bq)
            nc.sync.dma_start(bk_sb[:], bk)
        else:
            bq_sb = bk_sb = None

        # weight DMAs are emitted inside _kernel_body, ordered so the first
        # projection matmuls wait only on wq + the first xt quarter
        wp = ctx.enter_context(tc.tile_pool(name="w", bufs=1))
        wq_sb = wp.tile([P, CO, GW], BF16)
        wk_sb = wp.tile([P, CO, GW], BF16)
        wv_sb = wp.tile([P, CO, GW], BF16)
        wop = ctx.enter_context(tc.tile_pool(name="wo", bufs=1))
        wo_sb = wop.tile([P, NPAIR, D], BF16)

        qkp = ctx.enter_context(tc.tile_pool(name="qk", bufs=1))
        qT = qkp.tile([P, NPAIR, L], BF16)   # [d-in-pair, pair, l]
        kT = qkp.tile([P, NPAIR, L], BF16)
        vp = ctx.enter_context(tc.tile_pool(name="v", bufs=1))
        vext = vp.tile([P, LT, HPG, DK + 1], BF16)  # [l-in-tile, ltile, head, d|1]
        otp = ctx.enter_context(tc.tile_pool(name="ot", bufs=3))

        with tc.tile_pool(name="xt", bufs=3) as xtp, \
             tc.tile_pool(name="pp", bufs=2, space="PSUM") as ppp, \
             tc.tile_pool(name="ps", bufs=2, space="PSUM") as pss, \
             tc.tile_pool(name="po", bufs=1, space="PSUM") as pso, \
             tc.tile_pool(name="es", bufs=6) as esp, \
             tc.tile_pool(name="rc", bufs=3) as rcp, \
             tc.tile_pool(name="fy", bufs=1) as fyp, \
             tc.tile_pool(name="yb", bufs=3) as ybp:

            for _rep in range(repeat):
                _kernel_body(nc, tc, mm, with_qk_bias, locals(), ablate)

    nc.compile()
    return nc


def _kernel_body(nc, tc, mm, with_qk_bias, env, ablate=frozenset()):
    (qT, kT, vext, otp, mtri_sb, ones_sb, wq_sb, wk_sb, wv_sb, wo_sb,
     bq_sb, bk_sb) = (env[k] for k in (
         "qT", "kT", "vext", "otp", "mtri_sb", "ones_sb", "wq_sb", "wk_sb",
         "wv_sb", "wo_sb", "bq_sb", "bk_sb"))
    xt_r, y_r = env["xt_r"], env["y_r"]
    xtp, ppp, pss, pso, esp, rcp, fyp, ybp = (env[k] for k in (
        "xtp", "ppp", "pss", "pso", "esp", "rcp", "fyp", "ybp"))

    xt_tiles = {}

    def dma_quarter(k):
        xt_sb = xtp.tile([P, CO, QW], BF16, tag="xt")
        for co in range(CO):
            nc.sync.dma_start(xt_sb[:, co], xt_r[co, :, k * QW:(k + 1) * QW])
        xt_tiles[k] = xt_sb

    def proj_units(k):
        """QKV projection for l-quarter k as a list of emission thunks."""
        xt_sb = xt_tiles[k]
        units = []
        for w_sb, dest, b_sb in ((wq_sb, qT, bq_sb), (wk_sb, kT, bk_sb)):
            for pair in range(NPAIR):
                def qk_unit(w_sb=w_sb, dest=dest, b_sb=b_sb, pair=pair):
                    pt = ppp.tile([P, QW], F32, tag="pp")
                    for co in range(CO):
                        mm(pt[:], w_sb[:, co, pair * P:(pair + 1) * P],
                           xt_sb[:, co], start=co == 0, stop=co == CO - 1)
                    dsl = dest[:, pair, k * QW:(k + 1) * QW]
                    if with_qk_bias:
                        nc.vector.tensor_scalar_add(
                            dsl, pt[:], b_sb[:, pair:pair + 1])
                    else:
                        nc.vector.tensor_copy(dsl, pt[:])
                units.append(qk_unit)
        for i in range(4):
            def v_unit(i=i):
                lt = 4 * k + i
                pv = ppp.tile([P, GW], F32, tag="pp")
                for co in range(CO):
                    mm(pv[:], xt_sb[:, co, i * P:(i + 1) * P],
                       wv_sb[:, co], start=co == 0, stop=co == CO - 1)
                nc.vector.tensor_copy(
                    vext[:, lt, :, 0:DK],
                    pv[:].rearrange("p (h d) -> p h d", h=HPG))
                nc.vector.tensor_copy(
                    vext[:, lt, :, DK:DK + 1],
                    ones_sb[:, :, None].to_broadcast((P, HPG, 1)))
            units.append(v_unit)
        return units

    def out_proj_part(oT, qb, i):
        if "out" in ablate:
            return
        lt = 4 * qb + i
        yb = ybp.tile([P, D], F32, tag="yb")
        for eh in range(2):
            py = ppp.tile([P, QW], F32, tag="pp")
            for pair in range(NPAIR):
                mm(py[:], oT[:, pair, i * P:(i + 1) * P],
                   wo_sb[:, pair, eh * QW:(eh + 1) * QW],
                   start=pair == 0, stop=pair == NPAIR - 1)
            nc.vector.tensor_copy(yb[:, eh * QW:(eh + 1) * QW], py[:])
            # store each half as soon as it lands (shorter critical tail)
            nc.sync.dma_start(y_r[lt][:, eh * QW:(eh + 1) * QW],
                              yb[:, eh * QW:(eh + 1) * QW])

    # head: DMA only what the first matmuls need (wq + xt quarter 0), then
    # stream the remaining weights behind the first projection units
    wq_r, wk_r, wv_r, wo_r = (env[k] for k in ("wq_r", "wk_r", "wv_r", "wo_r"))
    # PE clock warmup: the tensor engine needs ~3us of continuous work to
    # reach its 2.4 GHz p-state. Run throwaway matmuls on a memset tile
    # while the head DMAs stream so the first real matmuls start warm.
    wtile = rcp.tile([P, P], BF16, tag="warm", name="wtile")
    nc.vector.memset(wtile[:], 1.0)
    wps = ppp.tile([P, QW], F32, tag="pp", name="wps")
    for _ in range(24):
        mm(wps[:, 0:P], wtile[:], wtile[:], start=True, stop=True)
    # interleave wq/xt per-co so the first co-accumulation starts after ~2
    # DMAs instead of after the full wq + xt quarter
    xt_sb0 = xtp.tile([P, CO, QW], BF16, tag="xt", name="xt_sb0")
    for co in range(CO):
        nc.sync.dma_start(wq_sb[:, co], wq_r[co])
        nc.sync.dma_start(xt_sb0[:, co], xt_r[co, :, 0:QW])
    xt_tiles[0] = xt_sb0
    units0 = proj_units(0)
    for u in units0[0:4]:       # Q units
        u()
    for co in range(CO):
        nc.sync.dma_start(wk_sb[:, co], wk_r[co])
    for u in units0[4:8]:       # K units
        u()
    for co in range(CO):
        nc.sync.dma_start(wv_sb[:, co], wv_r[co])
    for u in units0[8:12]:      # V units
        u()
    for pair in range(NPAIR):
        nc.sync.dma_start(wo_sb[:, pair], wo_r[pair])

    # ---- global software pipeline over (qb, pair, j) ----
    # scores are emitted 2 steps ahead (across pair and qb boundaries) so the
    # ACT exp chain never waits on filler matmuls (out-proj / next-quarter
    # projection) emitted at pair boundaries.
    oT_t: dict = {}
    po_t: dict = {}
    ps_t: dict = {}
    es_t: dict = {}
    units_t: dict = {}

    def nj_of(qb):
        return 4 * qb + 4

    seq = [(qb, pair, j)
           for qb in range(QB)
           for pair in range(NPAIR)
           for j in range(nj_of(qb))]

    def scores(qb, pair, j):
        if (qb, pair) not in po_t:
            po_t[(qb, pair)] = (None, None) if "av" in ablate else (
                pso.tile([DK + 1, QW], F32, tag="po0", name="po0"),
                pso.tile([DK + 1, QW], F32, tag="po1", name="po1"))
        o = j - 4 * qb
        w0 = max(o, 0) * P
        ps2 = pss.tile([P, 2, QW], F32, tag="ps")
        qs0 = qT[0:DK, pair, qb * QW + w0:(qb + 1) * QW]
        qs1 = qT[DK:P, pair, qb * QW + w0:(qb + 1) * QW]
        mm(ps2[:, 0, w0:QW], kT[0:DK, pair, j * P:(j + 1) * P],
           qs0, start=True, stop=True, tile_position=(0, 0))
        mm(ps2[:, 1, w0:QW], kT[DK:P, pair, j * P:(j + 1) * P],
           qs1, start=True, stop=True, tile_position=(64, 0))
        ps_t[(qb, pair, j)] = ps2

    es_const = [None]

    def expmask(qb, pair, j):
        o = j - 4 * qb
        w0 = max(o, 0) * P
        if "exp" in ablate:
            if es_const[0] is None:
                ec = esp.tile([P, 2, QW], BF16, tag="es", name="es_const")
                nc.any.memset(ec[:], 0.001)
                es_const[0] = ec
            es_t[(qb, pair, j)] = es_const[0]
            return
        es2 = esp.tile([P, 2, QW], BF16, tag="es")
        with tc.high_priority():
            # exp is the AV-gating chain: never let evac copies delay it
            nc.scalar.activation(es2[:, :, w0:QW],
                                 ps_t[(qb, pair, j)][:, :, w0:QW], AF.Exp)
        if o >= 0 and "mask" not in ablate:
            # causal triangle inside the diagonal 128-block; AV-gating like exp
            dj = es2[:, :, o * P:(o + 1) * P]
            with tc.high_priority():
                nc.vector.tensor_mul(
                    dj, dj, mtri_sb[:, None, :].to_broadcast((P, 2, P)))
        es_t[(qb, pair, j)] = es2

    def av(qb, pair, j):
        nj = nj_of(qb)
        o = j - 4 * qb
        w0 = max(o, 0) * P
        es2 = es_t.pop((qb, pair, j))
        ps_t.pop((qb, pair, j))
        po0, po1 = po_t[(qb, pair)]
        if "av" in ablate:
            return
        mm(po0[:, w0:QW], vext[:, j, 2 * pair, :], es2[:, 0, w0:QW],
           start=j == 0, stop=j == nj - 1)
        mm(po1[:, w0:QW], vext[:, j, 2 * pair + 1, :], es2[:, 1, w0:QW],
           start=j == 0, stop=j == nj - 1)

    def norm(qb, pair):
        # normalization: 1/sums broadcast, multiply straight from PSUM
        po0, po1 = po_t.pop((qb, pair))
        if "norm" in ablate:
            return
        oT = oT_t[qb]
        # sums row lives at partition 64; approx-recip needs base partition 0
        # (HW quirk) -> aligned cross-copy, then one batched reciprocal.
        # High priority: this chain frees the po banks the next pair's AV
        # needs, so it must jump the DVE queue ahead of evacuation copies.
        with tc.high_priority():
            sm = rcp.tile([1, 2, QW], F32, tag="sm")
            nc.vector.tensor_copy(sm[:, 0, :], po0[DK:DK + 1, :])
            nc.vector.tensor_copy(sm[:, 1, :], po1[DK:DK + 1, :])
            rc = rcp.tile([1, 2, QW], F32, tag="rc")
            nc.vector.reciprocal_approx_fast(rc[:], sm[:])
            rcb = rcp.tile([DK, 2, QW], F32, tag="rcb")
            nc.gpsimd.partition_broadcast(rcb[:], rc[:])
            nc.vector.tensor_mul(oT[0:DK, pair, :], po0[0:DK, :], rcb[:, 0, :])
            nc.vector.tensor_mul(oT[DK:P, pair, :], po1[0:DK, :], rcb[:, 1, :])

    for idx, (qb, pair, j) in enumerate(seq):
        if pair == 0 and j == 0:
            # q-block entry: O^T accumulator, next quarter's xt DMA + units
            oT_t[qb] = otp.tile([P, NPAIR, QW], BF16, tag="ot",
                                name=f"oT{qb}")
            if "norm" in ablate and "out" not in ablate:
                nc.gpsimd.memset(oT_t[qb][:], 0.0)
            if qb + 1 < QB:
                dma_quarter(qb + 1)
                units_t[qb] = proj_units(qb + 1)
            else:
                units_t[qb] = []
            if idx == 0:
                scores(*seq[0])
                scores(*seq[1])
                scores(*seq[2])
                scores(*seq[3])
        expmask(qb, pair, j)
        if idx + 4 < len(seq):
            scores(*seq[idx + 4])
        av(qb, pair, j)
        if j == nj_of(qb) - 1:  # pair finished: norm + fillers
            norm(qb, pair)
            if qb > 0:
                out_proj_part(oT_t[qb - 1], qb - 1, pair)
            for u in units_t[qb][3 * pair:3 * pair + 3]:
                u()
            if pair == NPAIR - 1 and qb > 0:
                del oT_t[qb - 1]
    for i in range(4):
        out_proj_part(oT_t[QB - 1], QB - 1, i)


def _prep_inputs(x, W_qkv, b_qkv, W_out):
    """Per-core input maps. Core c -> batch c//2, head-group c%2."""
    import ml_dtypes
    bf16 = ml_dtypes.bfloat16
    x = np.asarray(x, dtype=np.float32)
    W_qkv = np.asarray(W_qkv, dtype=np.float32)
    b_qkv = np.asarray(b_qkv, dtype=np.float32)
    W_out = np.asarray(W_out, dtype=np.float32)

    scale = 1.0 / np.sqrt(DK)
    mtri = (np.arange(P)[None, :] >= np.arange(P)[:, None]).astype(bf16)
    onesd = np.ones((P, 1), dtype=bf16)

    with_qk_bias = bool(np.any(b_qkv[:2 * D]))
    xts = [np.ascontiguousarray(x[b].T).astype(bf16) for b in range(B)]
    in_maps = []
    for c in range(8):
        b, g = c // 2, c % 2
        sl = slice(g * GW, (g + 1) * GW)
        m = {
            "xt": xts[b],
            "wq": np.ascontiguousarray(
                W_qkv[:, g * GW:(g + 1) * GW] * scale).astype(bf16),
            "wk": np.ascontiguousarray(
                W_qkv[:, D + g * GW:D + (g + 1) * GW]).astype(bf16),
            "wv": np.ascontiguousarray(
                W_qkv[:, 2 * D + g * GW:2 * D + (g + 1) * GW]).astype(bf16),
            "wo": np.ascontiguousarray(W_out[sl, :]).astype(bf16),
            "mtri": mtri,
            "onesd": onesd,
        }
        if with_qk_bias:
            m["bq"] = np.ascontiguousarray(
                b_qkv[g * GW:(g + 1) * GW].reshape(NPAIR, P).T) * scale
            m["bk"] = np.ascontiguousarray(
                b_qkv[D + g * GW:D + (g + 1) * GW].reshape(NPAIR, P).T)
        in_maps.append(m)
    return in_maps, with_qk_bias


def kernel(x, W_qkv, b_qkv, W_out, b_out):
    in_maps, with_qk_bias = _prep_inputs(x, W_qkv, b_qkv, W_out)

    key = ("nc", with_qk_bias)
    if key not in _NC_CACHE:
        _NC_CACHE[key] = build_nc(with_qk_bias)
    nc = _NC_CACHE[key]

    res = bass_utils.run_bass_kernel_spmd(nc, in_maps, core_ids=list(range(8)))
    parts = [r["y"] for r in res.results]

    b_qkv = np.asarray(b_qkv, dtype=np.float32)
    W_out_np = np.asarray(W_out, dtype=np.float32)
    # v-bias passes through attention (rows of attn sum to 1) and out-proj is
    # linear: contribution = b_v @ W_out; b_out adds directly.
    corr = (b_qkv[2 * D:3 * D] @ W_out_np
            + np.asarray(b_out, dtype=np.float32)).astype(np.float32)

    out = np.empty((B, L, D), dtype=np.float32)
    for b in range(B):
        out[b] = parts[2 * b] + parts[2 * b + 1] + corr
    return out
